# revision 1
# baseline (speedup 1.0000x reference)
import os
import sys
from contextlib import ExitStack

import numpy as np

sys.path.insert(0, "/opt/trn_rl_repo")

import concourse.bass as bass
from concourse import bacc
import concourse.tile as tile
from concourse import mybir
from concourse.bass_utils import run_bass_kernel_spmd

# Problem constants (hardcoded per contract)
B, T, N, F_IN, F_OUT = 64, 12, 325, 32, 128
NC = 8          # cores
BL = B // NC    # batch per core = 8
NP = 384        # padded node count (3 x 128)
NJ = 3          # node chunks
CX = F_IN + 1   # x channels + ones channel (bias trick) = 33
CH = F_OUT      # 128
NOPS = 5        # I, A_out, A_in, A_out2, A_in2
F32 = mybir.dt.float32

_CACHE = {}


def _build_bass():
    nc = bacc.Bacc(None, target_bir_lowering=False)
    x_d = nc.dram_tensor("xin", [128, NJ, T, BL, CX], F32, kind="ExternalInput")
    a_d = nc.dram_tensor("amat", [128, NJ, NOPS, NP], F32, kind="ExternalInput")
    wzrx_d = nc.dram_tensor("wzrx", [CX, NOPS, 2 * F_OUT], F32, kind="ExternalInput")
    wzrh_d = nc.dram_tensor("wzrh", [CH, NOPS, 2 * F_OUT], F32, kind="ExternalInput")
    whx_d = nc.dram_tensor("whx", [CX, NOPS, F_OUT], F32, kind="ExternalInput")
    whh_d = nc.dram_tensor("whh", [CH, NOPS, F_OUT], F32, kind="ExternalInput")
    y_d = nc.dram_tensor("y", [128, T, NJ, BL, F_OUT], F32, kind="ExternalOutput")

    with tile.TileContext(nc) as tc, ExitStack() as ctx:
        const = ctx.enter_context(tc.tile_pool(name="const", bufs=1))
        state = ctx.enter_context(tc.tile_pool(name="state", bufs=1))
        gpool = ctx.enter_context(tc.tile_pool(name="g", bufs=3))
        gcp = ctx.enter_context(tc.tile_pool(name="gcp", bufs=2))
        mid = ctx.enter_context(tc.tile_pool(name="mid", bufs=2))
        spool = ctx.enter_context(tc.tile_pool(name="s", bufs=3))
        psg = ctx.enter_context(tc.tile_pool(name="psg", bufs=2, space="PSUM"))
        psx = ctx.enter_context(tc.tile_pool(name="psx", bufs=2, space="PSUM"))
        psz = ctx.enter_context(tc.tile_pool(name="psz", bufs=2, space="PSUM"))
        psc = ctx.enter_context(tc.tile_pool(name="psc", bufs=2, space="PSUM"))

        xin = const.tile([128, NJ, T, BL, CX], F32)
        amat = const.tile([128, NJ, NOPS, NP], F32)
        wx = const.tile([CX, NOPS, 3 * F_OUT], F32)
        wh = const.tile([CH, NOPS, 3 * F_OUT], F32)
        nc.sync.dma_start(xin[:], x_d[:])
        nc.sync.dma_start(amat[:], a_d[:])
        nc.sync.dma_start(wx[:, :, 0:2 * F_OUT], wzrx_d[:])
        nc.sync.dma_start(wh[:, :, 0:2 * F_OUT], wzrh_d[:])
        nc.sync.dma_start(wx[:, :, 2 * F_OUT:], whx_d[:])
        nc.sync.dma_start(wh[:, :, 2 * F_OUT:], whh_d[:])

        hs = state.tile([128, NJ, BL, CH], F32)  # node-major hidden state
        nc.gpsimd.memset(hs[:], 0.0)

        def graph_ops(lhs_fn, cpart, gtile, ps_pool):
            # gtile[c, P, d] = sum_s lhs[s, c] * amat[s, P, d]  (channel-major result)
            for P in range(NOPS):
                ps = ps_pool.tile([cpart, NP], F32)
                for j in range(NJ):
                    nc.tensor.matmul(ps[:], lhs_fn(j), amat[:, j, P, :],
                                     start=(j == 0), stop=(j == NJ - 1))
                nc.scalar.copy(gtile[:, P, :], ps[:])

        FO = F_OUT

        def zr_graph(t, b, gs):
            gx = gpool.tile([CX, NOPS, NP], F32, tag="gx")
            graph_ops(lambda j: xin[:, j, t, b, :], CX, gx, psx)
            gh = gpool.tile([CH, NOPS, NP], F32, tag="gh")
            graph_ops(lambda j: hs[:, j, b, :], CH, gh, psg)
            gs[b] = (gx, gh)

        def zr_gates(b, gs, zs):
            gx, gh = gs[b]
            zt = mid.tile([128, NJ, FO], F32, tag="zt")
            hr = mid.tile([128, NJ, CH], F32, tag="hr")
            for m in range(NJ):
                pz = psz.tile([128, 2 * FO], F32)
                for P in range(NOPS):
                    nc.tensor.matmul(pz[:], gx[:, P, bass.ts(m, 128)],
                                     wx[:, P, 0:2 * FO], start=(P == 0), stop=False)
                for P in range(NOPS):
                    nc.tensor.matmul(pz[:], gh[:, P, bass.ts(m, 128)],
                                     wh[:, P, 0:2 * FO], start=False,
                                     stop=(P == NOPS - 1))
                nc.scalar.activation(zt[:, m, :], pz[:, 0:FO],
                                     mybir.ActivationFunctionType.Sigmoid)
                rt = spool.tile([128, FO], F32, tag="tmp")
                nc.scalar.activation(rt[:], pz[:, FO:2 * FO],
                                     mybir.ActivationFunctionType.Sigmoid)
                nc.vector.tensor_mul(hr[:, m, :], hs[:, m, b, :], rt[:])
            zs[b] = (zt, hr)

        def cand_graph(b, zs, cs):
            zt, hr = zs[b]
            gc = gcp.tile([CH, NOPS, NP], F32, tag="gc")
            graph_ops(lambda j: hr[:, j, :], CH, gc, psg)
            cs[b] = gc

        def cand_gates(t, b, gs, zs, cs):
            gx, _ = gs[b]
            zt, _ = zs[b]
            gc = cs[b]
            for m in range(NJ):
                pc = psc.tile([128, FO], F32)
                for P in range(NOPS):
                    nc.tensor.matmul(pc[:], gx[:, P, bass.ts(m, 128)],
                                     wx[:, P, 2 * FO:], start=(P == 0), stop=False)
                for P in range(NOPS):
                    nc.tensor.matmul(pc[:], gc[:, P, bass.ts(m, 128)],
                                     wh[:, P, 2 * FO:], start=False,
                                     stop=(P == NOPS - 1))
                ht = spool.tile([128, FO], F32, tag="tmp")
                nc.scalar.activation(ht[:], pc[:],
                                     mybir.ActivationFunctionType.Tanh)
                d1 = spool.tile([128, FO], F32, tag="tmp")
                nc.vector.tensor_sub(d1[:], hs[:, m, b, :], ht[:])
                d2 = spool.tile([128, FO], F32, tag="tmp")
                nc.vector.tensor_mul(d2[:], zt[:, m, :], d1[:])
                nc.vector.tensor_add(hs[:, m, b, :], ht[:], d2[:])
            nc.sync.dma_start(y_d[:, t, :, b, :], hs[:, :, b, :])

        for t in range(T):
            gs, zs, cs = {}, {}, {}
            zr_graph(t, 0, gs)
            zr_graph(t, 1, gs)
            zr_gates(0, gs, zs)
            for b in range(BL):
                if b + 2 < BL:
                    zr_graph(t, b + 2, gs)
                cand_graph(b, zs, cs)
                if b + 1 < BL:
                    zr_gates(b + 1, gs, zs)
                cand_gates(t, b, gs, zs, cs)
    nc.compile()
    return nc


def _prep_consts(edge_index, edge_weight, Wz, bz, Wr, br, Wh, bh):
    row = edge_index[0].astype(np.int64)
    col = edge_index[1].astype(np.int64)
    w = edge_weight.astype(np.float32)
    deg_out = np.zeros(N, np.float32)
    deg_in = np.zeros(N, np.float32)
    np.add.at(deg_out, row, w)
    np.add.at(deg_in, col, w)
    norm_out = (1.0 / deg_out)[row]
    norm_in = (1.0 / deg_in)[row]  # quirk: indexed by row
    perm = np.argsort(col * N + row, kind="stable")
    A_out = np.zeros((N, N), np.float32)
    A_in = np.zeros((N, N), np.float32)
    np.add.at(A_out, (col, row), norm_out)
    np.add.at(A_in, (row[perm], col[perm]), norm_in)  # norm_in unpermuted
    I = np.eye(N, dtype=np.float32)
    A_out2 = 2.0 * (A_out @ A_out) - I
    A_in2 = 2.0 * (A_in @ A_in) - I

    amat = np.zeros((NOPS, NP, NP), np.float32)  # [P, d, s]
    for i, A in enumerate([I, A_out, A_in, A_out2, A_in2]):
        amat[i, :N, :N] = A
    # rhs layout [s%128, j, P, d]: AT[P][s, d] = A[d, s]
    amat_r = amat.transpose(2, 0, 1).reshape(NJ, 128, NOPS, NP).transpose(1, 0, 2, 3)
    amat_r = np.ascontiguousarray(amat_r)

    def terms(W):  # W: [2, 3, C, co] -> list of 5 [C, co]
        return [W[0, 0] + W[1, 0], W[0, 1], W[1, 1], W[0, 2], W[1, 2]]

    tz, tr, th = terms(Wz), terms(Wr), terms(Wh)
    wzrx = np.zeros((CX, NOPS, 2 * F_OUT), np.float32)
    wzrh = np.zeros((CH, NOPS, 2 * F_OUT), np.float32)
    whx = np.zeros((CX, NOPS, F_OUT), np.float32)
    whh = np.zeros((CH, NOPS, F_OUT), np.float32)
    for P in range(NOPS):
        wzr = np.concatenate([tz[P], tr[P]], axis=1)  # [C, 256]
        wzrx[:F_IN, P] = wzr[:F_IN]
        wzrh[:, P] = wzr[F_IN:]
        whx[:F_IN, P] = th[P][:F_IN]
        whh[:, P] = th[P][F_IN:]
    wzrx[F_IN, 0] = np.concatenate([bz, br])  # bias via ones channel, op I only
    whx[F_IN, 0] = bh
    return amat_r, wzrx, wzrh, whx, whh


def kernel(X, edge_index, edge_weight, Wz, bz, Wr, br, Wh, bh):
    X = np.asarray(X, np.float32)
    amat_r, wzrx, wzrh, whx, whh = _prep_consts(
        np.asarray(edge_index), np.asarray(edge_weight, np.float32),
        np.asarray(Wz, np.float32), np.asarray(bz, np.float32),
        np.asarray(Wr, np.float32), np.asarray(br, np.float32),
        np.asarray(Wh, np.float32), np.asarray(bh, np.float32))

    if "nc" not in _CACHE:
        _CACHE["nc"] = _build_bass()
    nc = _CACHE["nc"]

    in_maps = []
    for c in range(NC):
        Xl = X[c * BL:(c + 1) * BL]  # [BL, T, N, F_IN]
        Xp = np.zeros((BL, T, NP, CX), np.float32)
        Xp[:, :, :N, :F_IN] = Xl
        Xp[:, :, :, F_IN] = 1.0
        # -> [p, j, t, b, c]
        Xp = Xp.reshape(BL, T, NJ, 128, CX).transpose(3, 2, 1, 0, 4)
        in_maps.append({
            "xin": np.ascontiguousarray(Xp),
            "amat": amat_r, "wzrx": wzrx, "wzrh": wzrh,
            "whx": whx, "whh": whh,
        })

    trace = bool(int(os.environ.get("KERNEL_TRACE", "0")))
    res = run_bass_kernel_spmd(nc, in_maps, core_ids=list(range(NC)), trace=trace)
    _CACHE["last_result"] = res

    out = np.empty((B, T, N, F_OUT), np.float32)
    for c in range(NC):
        y = res.results[c]["y"]  # [128, T, NJ, BL, F_OUT]
        y = y.reshape(128, T, NJ, BL, F_OUT).transpose(3, 1, 2, 0, 4)
        out[c * BL:(c + 1) * BL] = y.reshape(BL, T, NP, F_OUT)[:, :, :N, :]
    return out



# revision 2
# speedup vs baseline: 4.2390x; 4.2390x over previous
import os
import sys
from contextlib import ExitStack

import numpy as np
import ml_dtypes

sys.path.insert(0, "/opt/trn_rl_repo")

import concourse.bass as bass
from concourse import bacc
import concourse.tile as tile
from concourse import mybir
from concourse.bass_utils import run_bass_kernel_spmd

# Problem constants (hardcoded per contract)
B, T, N, F_IN, F_OUT = 64, 12, 325, 32, 128
NC = 8          # cores
BL = B // NC    # batch per core = 8
NJ = 3          # node chunks
ND = 325        # graph output free dim (true node count, no padding)
# Overlapping node chunks: {0:128, 128:256, 197:325}. Chunk 2 rows that
# duplicate chunk 1 (nodes 197..255) are zeroed in the operator so the
# contraction over chunks stays exact, while every matmul output keeps a
# full 128-partition write (no stale-PSUM reads downstream).
CHUNK0 = [0, 128, 197]
CX = F_IN + 1   # x channels + ones channel (bias trick) = 33
CH = F_OUT      # 128
NOPS = 5        # I, A_out, A_in, A_out2, A_in2
F32 = mybir.dt.float32
BF16 = mybir.dt.bfloat16
SIG = mybir.ActivationFunctionType.Sigmoid
TANH = mybir.ActivationFunctionType.Tanh

_CACHE = {}


def _build_bass():
    nc = bacc.Bacc(None, target_bir_lowering=False)
    x_d = nc.dram_tensor("xin", [128, NJ, T, BL, CX], BF16, kind="ExternalInput")
    a_d = nc.dram_tensor("amat", [128, NJ, NOPS, ND], BF16, kind="ExternalInput")
    wx_d = nc.dram_tensor("wx", [CX, NOPS, 3 * F_OUT], BF16, kind="ExternalInput")
    wh_d = nc.dram_tensor("wh", [CH, NOPS, 3 * F_OUT], BF16, kind="ExternalInput")
    y_d = nc.dram_tensor("y", [128, T, BL, NJ * F_OUT], BF16, kind="ExternalOutput")

    with tile.TileContext(nc) as tc, ExitStack() as ctx:
        const = ctx.enter_context(tc.tile_pool(name="const", bufs=1))
        state = ctx.enter_context(tc.tile_pool(name="state", bufs=1))
        gpool = ctx.enter_context(tc.tile_pool(name="g", bufs=3))
        gxpool = ctx.enter_context(tc.tile_pool(name="gx", bufs=3))
        gcp = ctx.enter_context(tc.tile_pool(name="gcp", bufs=2))
        mid = ctx.enter_context(tc.tile_pool(name="mid", bufs=3))
        spool = ctx.enter_context(tc.tile_pool(name="s", bufs=3))
        psg = ctx.enter_context(tc.tile_pool(name="psg", bufs=4, space="PSUM"))
        psgate = ctx.enter_context(tc.tile_pool(name="psgate", bufs=4, space="PSUM"))

        xin = const.tile([128, NJ, T, BL, CX], BF16)
        amat = const.tile([128, NJ, NOPS, ND], BF16)
        wx = const.tile([CX, NOPS, 3 * F_OUT], BF16)
        wh = const.tile([CH, NOPS, 3 * F_OUT], BF16)
        nc.sync.dma_start(xin[:], x_d[:])
        nc.sync.dma_start(amat[:], a_d[:])
        nc.sync.dma_start(wx[:], wx_d[:])
        nc.sync.dma_start(wh[:], wh_d[:])

        hs = state.tile([128, BL, NJ, CH], BF16)  # node-major hidden state
        nc.gpsimd.memset(hs[:], 0.0)

        FO = F_OUT
        # node-chunk slices of the graph-op output (free dim of g tiles)
        mslc = [slice(s, s + 128) for s in CHUNK0]

        def graph_ops(lhs_fn, cpart, gtile, evict_engines):
            # gtile[c, P, d] = sum_s lhs[s, c] * amat[s, P, d]
            for P in range(NOPS):
                ps = psg.tile([128, ND], F32, tag="ps")
                for j in range(NJ):
                    nc.tensor.matmul(ps[0:cpart, :], lhs_fn(j), amat[:, j, P, :],
                                     start=(j == 0), stop=(j == NJ - 1))
                eng = evict_engines[P]
                if eng == "act":
                    nc.scalar.copy(gtile[:, P, :], ps[0:cpart, :])
                else:
                    nc.vector.tensor_copy(gtile[:, P, :], ps[0:cpart, :])

        def zr_graph(t, b, gs):
            gx = gxpool.tile([CX, NOPS, ND], BF16, tag="gx")
            graph_ops(lambda j: xin[:, j, t, b, :], CX, gx,
                      ["act", "dve", "act", "dve", "act"])
            gh = gpool.tile([CH, NOPS, ND], BF16, tag="gh")
            graph_ops(lambda j: hs[:, b, j, :], CH, gh,
                      ["dve", "act", "dve", "act", "dve"])
            gs[b] = (gx, gh)

        def zr_gates(b, gs, zs):
            gx, gh = gs[b]
            zrt = mid.tile([128, NJ, 2 * FO], BF16, tag="zrt")
            pzs = []
            for m in range(NJ):
                pz = psgate.tile([128, 3 * FO], F32, tag="pz")
                pzs.append(pz)
                # x-part covers all three gates (z|r|cand) in one rhs
                for P in range(NOPS):
                    nc.tensor.matmul(pz[:], gx[:, P, mslc[m]], wx[:, P, :],
                                     start=(P == 0), stop=False)
                # h-part for z,r only; stop closes cols 0:256 for the sigmoid
                for P in range(NOPS):
                    nc.tensor.matmul(pz[:, 0:2 * FO], gh[:, P, mslc[m]],
                                     wh[:, P, 0:2 * FO], start=False,
                                     stop=(P == NOPS - 1),
                                     skip_group_check=(P == NOPS - 1))
                nc.scalar.activation(zrt[:, m, :], pz[:, 0:2 * FO], SIG)
            hrt = mid.tile([128, NJ, CH], BF16, tag="hrt")
            nc.vector.tensor_mul(hrt[:], hs[:, b, :, :], zrt[:, :, FO:2 * FO])
            zs[b] = (zrt, hrt, pzs)

        def cand_graph(b, zs, cs):
            _, hrt, _ = zs[b]
            gc = gcp.tile([CH, NOPS, ND], BF16, tag="gc")
            graph_ops(lambda j: hrt[:, j, :], CH, gc,
                      ["dve", "act", "dve", "act", "dve"])
            cs[b] = gc

        def cand_gates(t, b, gs, zs, cs):
            zrt, _, pzs = zs[b]
            gc = cs[b]
            ht = mid.tile([128, NJ, FO], BF16, tag="ht")
            for m in range(NJ):
                pz = pzs[m]
                for P in range(NOPS):
                    nc.tensor.matmul(pz[:, 2 * FO:], gc[:, P, mslc[m]],
                                     wh[:, P, 2 * FO:], start=False,
                                     stop=(P == NOPS - 1))
                nc.scalar.activation(ht[:, m, :], pz[:, 2 * FO:], TANH)
            d1 = spool.tile([128, NJ, FO], BF16, tag="d1")
            nc.vector.tensor_sub(d1[:], hs[:, b, :, :], ht[:])
            d2 = spool.tile([128, NJ, FO], BF16, tag="d2")
            nc.vector.tensor_mul(d2[:], zrt[:, :, 0:FO], d1[:])
            nc.vector.tensor_add(hs[:, b, :, :], ht[:], d2[:])
            nc.sync.dma_start(y_d[:, t, b, :], hs[:, b, :, :])

        for t in range(T):
            gs, zs, cs = {}, {}, {}
            zr_graph(t, 0, gs)
            zr_graph(t, 1, gs)
            zr_gates(0, gs, zs)
            for b in range(BL):
                if b + 2 < BL:
                    zr_graph(t, b + 2, gs)
                cand_graph(b, zs, cs)
                if b + 1 < BL:
                    zr_gates(b + 1, gs, zs)
                cand_gates(t, b, gs, zs, cs)
    nc.compile()
    return nc


def _prep_consts(edge_index, edge_weight, Wz, bz, Wr, br, Wh, bh):
    row = edge_index[0].astype(np.int64)
    col = edge_index[1].astype(np.int64)
    w = edge_weight.astype(np.float32)
    deg_out = np.zeros(N, np.float32)
    deg_in = np.zeros(N, np.float32)
    np.add.at(deg_out, row, w)
    np.add.at(deg_in, col, w)
    norm_out = (1.0 / deg_out)[row]
    norm_in = (1.0 / deg_in)[row]  # quirk: indexed by row
    perm = np.argsort(col * N + row, kind="stable")
    A_out = np.zeros((N, N), np.float32)
    A_in = np.zeros((N, N), np.float32)
    np.add.at(A_out, (col, row), norm_out)
    np.add.at(A_in, (row[perm], col[perm]), norm_in)  # norm_in unpermuted
    I = np.eye(N, dtype=np.float32)
    A_out2 = 2.0 * (A_out @ A_out) - I
    A_in2 = 2.0 * (A_in @ A_in) - I

    # rhs layout [s%128, j, P, d]: value = A_P[d, s] for s = CHUNK0[j] + srow.
    # Chunk 2 rows 0:59 (s in 197..255) duplicate chunk 1 -> zeroed.
    amat_r = np.zeros((128, NJ, NOPS, ND), np.float32)
    for i, A in enumerate([I, A_out, A_in, A_out2, A_in2]):
        for j, s0 in enumerate(CHUNK0):
            blk = A[:, s0:s0 + 128].T.copy()  # [128 srow, ND]
            if j == 2:
                blk[0:256 - 197] = 0.0
            amat_r[:, j, i, :] = blk

    def terms(W):  # W: [2, 3, C, co] -> list of 5 [C, co]
        return [W[0, 0] + W[1, 0], W[0, 1], W[1, 1], W[0, 2], W[1, 2]]

    tz, tr, th = terms(Wz), terms(Wr), terms(Wh)
    wx = np.zeros((CX, NOPS, 3 * F_OUT), np.float32)
    wh = np.zeros((CH, NOPS, 3 * F_OUT), np.float32)
    for P in range(NOPS):
        cat = np.concatenate([tz[P], tr[P], th[P]], axis=1)  # [C, 384]
        wx[:F_IN, P] = cat[:F_IN]
        wh[:, P] = cat[F_IN:]
    wx[F_IN, 0] = np.concatenate([bz, br, bh])  # bias via ones channel, op I only
    amat_r = amat_r.astype(ml_dtypes.bfloat16)
    wx = wx.astype(ml_dtypes.bfloat16)
    wh = wh.astype(ml_dtypes.bfloat16)
    return np.ascontiguousarray(amat_r), wx, wh


def kernel(X, edge_index, edge_weight, Wz, bz, Wr, br, Wh, bh):
    X = np.asarray(X, np.float32)
    amat_r, wx, wh = _prep_consts(
        np.asarray(edge_index), np.asarray(edge_weight, np.float32),
        np.asarray(Wz, np.float32), np.asarray(bz, np.float32),
        np.asarray(Wr, np.float32), np.asarray(br, np.float32),
        np.asarray(Wh, np.float32), np.asarray(bh, np.float32))

    if "nc" not in _CACHE:
        _CACHE["nc"] = _build_bass()
    nc = _CACHE["nc"]

    in_maps = []
    for c in range(NC):
        Xl = X[c * BL:(c + 1) * BL]  # [BL, T, N, F_IN]
        Xp = np.zeros((BL, T, 128, NJ, CX), np.float32)
        for j, s0 in enumerate(CHUNK0):
            Xp[:, :, :, j, :F_IN] = Xl[:, :, s0:s0 + 128, :]
        Xp[:, :, :, :, F_IN] = 1.0
        # -> [p, j, t, b, c]
        Xp = Xp.transpose(2, 3, 1, 0, 4)
        in_maps.append({
            "xin": np.ascontiguousarray(Xp).astype(ml_dtypes.bfloat16),
            "amat": amat_r, "wx": wx, "wh": wh,
        })

    trace = bool(int(os.environ.get("KERNEL_TRACE", "0")))
    res = run_bass_kernel_spmd(nc, in_maps, core_ids=list(range(NC)), trace=trace)
    _CACHE["last_result"] = res

    out = np.empty((B, T, N, F_OUT), np.float32)
    for c in range(NC):
        y = res.results[c]["y"].astype(np.float32)  # [128, T, BL, NJ*F_OUT]
        y = y.reshape(128, T, BL, NJ, F_OUT).transpose(2, 1, 3, 0, 4)
        # [BL, T, NJ, 128, F_OUT]: nodes 0:128 | 128:256 | 256:325 (chunk2 rows 59:)
        blk = out[c * BL:(c + 1) * BL]
        blk[:, :, 0:128] = y[:, :, 0]
        blk[:, :, 128:256] = y[:, :, 1]
        blk[:, :, 256:325] = y[:, :, 2, 256 - 197:, :]
    return out


# revision 11
# speedup vs baseline: 5.3307x; 1.2575x over previous
import os
import sys
from contextlib import ExitStack

import numpy as np
import ml_dtypes

sys.path.insert(0, "/opt/trn_rl_repo")

import concourse.bass as bass
from concourse import bacc
import concourse.tile as tile
from concourse import mybir
from concourse.bass_utils import run_bass_kernel_spmd

# Problem constants (hardcoded per contract)
B, T, N, F_IN, F_OUT = 64, 12, 325, 32, 128
NC = 8          # cores
BL = B // NC    # batch per core = 8
NJ = 3          # node chunks
ND = 325        # graph output free dim (true node count, no padding)
# Overlapping node chunks: {0:128, 128:256, 197:325}. Chunk 2 rows that
# duplicate chunk 1 (nodes 197..255) are zeroed in the operator so the
# contraction over chunks stays exact, while every matmul output keeps a
# full 128-partition write (no stale-PSUM reads downstream).
CHUNK0 = [0, 128, 197]
CX = F_IN       # x channels = 32; bias rides as a packed ones-row
NG = 2          # batch groups of 4 for the packed x graph ops
CH = F_OUT      # 128
# x-side gate weights are (P, c)-packed along partitions:
#   pack1 rows: P in {0,1,2} x 32 channels = 96, plus ones/bias row -> 97
#   pack2 rows: P in {3,4} x 32 channels = 64
KP1, KP2 = 3 * F_IN + 1, 2 * F_IN
NOPS = 5        # I, A_out, A_in, A_out2, A_in2
F32 = mybir.dt.float32
BF16 = mybir.dt.bfloat16
SIG = mybir.ActivationFunctionType.Sigmoid
TANH = mybir.ActivationFunctionType.Tanh

_CACHE = {}


def _build_bass():
    nc = bacc.Bacc(None, target_bir_lowering=False)
    x_d = nc.dram_tensor("xin", [128, NJ, T, NG, 128], BF16, kind="ExternalInput")
    a_d = nc.dram_tensor("amat", [128, NJ, NOPS, ND], BF16, kind="ExternalInput")
    wx1_d = nc.dram_tensor("wx1", [KP1, 3 * F_OUT], BF16, kind="ExternalInput")
    wx2_d = nc.dram_tensor("wx2", [KP2, 3 * F_OUT], BF16, kind="ExternalInput")
    wh_d = nc.dram_tensor("wh", [CH, NOPS, 3 * F_OUT], BF16, kind="ExternalInput")
    y_d = nc.dram_tensor("y", [128, T, BL, NJ * F_OUT], BF16, kind="ExternalOutput")

    with tile.TileContext(nc) as tc, ExitStack() as ctx:
        const = ctx.enter_context(tc.tile_pool(name="const", bufs=1))
        state = ctx.enter_context(tc.tile_pool(name="state", bufs=1))
        gpool = ctx.enter_context(tc.tile_pool(name="g", bufs=3))
        gxpool = ctx.enter_context(tc.tile_pool(name="gx", bufs=6))
        gcp = ctx.enter_context(tc.tile_pool(name="gcp", bufs=2))
        mid = ctx.enter_context(tc.tile_pool(name="mid", bufs=3))
        spool = ctx.enter_context(tc.tile_pool(name="s", bufs=3))
        psg = ctx.enter_context(tc.tile_pool(name="psg", bufs=4, space="PSUM"))
        psgate = ctx.enter_context(tc.tile_pool(name="psgate", bufs=4, space="PSUM"))

        xin = const.tile([128, NJ, T, NG, 128], BF16)
        amat = const.tile([128, NJ, NOPS, ND], BF16)
        wx1 = const.tile([KP1, 3 * F_OUT], BF16)
        wx2 = const.tile([KP2, 3 * F_OUT], BF16)
        wh = const.tile([CH, NOPS, 3 * F_OUT], BF16)
        nc.sync.dma_start(xin[:], x_d[:])
        nc.sync.dma_start(amat[:], a_d[:])
        nc.sync.dma_start(wx1[:], wx1_d[:])
        nc.sync.dma_start(wx2[:], wx2_d[:])
        nc.sync.dma_start(wh[:], wh_d[:])

        hs = state.tile([128, BL, NJ, CH], BF16)  # node-major hidden state
        nc.gpsimd.memset(hs[:], 0.0)

        FO = F_OUT
        # node-chunk slices of the graph-op output (free dim of g tiles)
        mslc = [slice(s, s + 128) for s in CHUNK0]

        def graph_ops(lhs_fn, cpart, gtile, evict_engines):
            # gtile[c, P, d] = sum_s lhs[s, c] * amat[s, P, d]
            for P in range(NOPS):
                ps = psg.tile([128, ND], F32, tag="ps")
                for j in range(NJ):
                    nc.tensor.matmul(ps[0:cpart, :], lhs_fn(j), amat[:, j, P, :],
                                     start=(j == 0), stop=(j == NJ - 1))
                eng = evict_engines[P]
                if eng == "act":
                    nc.scalar.copy(gtile[:, P, :], ps[0:cpart, :])
                else:
                    nc.vector.tensor_copy(gtile[:, P, :], ps[0:cpart, :])

        def gx_graph(t, g, gxs):
            # packed x graph ops for a group of 4 batches: lhsT carries
            # (4 batches x 32 channels) on its free dim, so the PE runs with
            # all 128 output partitions live.
            packs = []
            for bb in range(4):
                g1 = gxpool.tile([KP1, ND], BF16, tag="gxp1")
                nc.gpsimd.memset(g1[3 * F_IN:KP1, :], 1.0)  # ones/bias row
                g2 = gxpool.tile([KP2, ND], BF16, tag="gxp2")
                packs.append((g1, g2))
                gxs[4 * g + bb] = (g1, g2)
            for P in range(NOPS):
                ps = psg.tile([128, ND], F32, tag="ps")
                for j in range(NJ):
                    nc.tensor.matmul(ps[:], xin[:, j, t, g, :], amat[:, j, P, :],
                                     start=(j == 0), stop=(j == NJ - 1))
                for bb in range(4):
                    g1, g2 = packs[bb]
                    src = ps[32 * bb:32 * bb + 32, :]
                    if P < 3:
                        dst = g1[32 * P:32 * P + 32, :]
                    else:
                        dst = g2[32 * (P - 3):32 * (P - 3) + 32, :]
                    if (P + bb) % 2 == 0:
                        nc.scalar.copy(dst, src)
                    else:
                        nc.vector.tensor_copy(dst, src)

        def zr_graph(t, b, gs):
            gh = gpool.tile([CH, NOPS, ND], BF16, tag="gh")
            graph_ops(lambda j: hs[:, b, j, :], CH, gh,
                      ["dve", "act", "dve", "act", "dve"])
            gs[b] = gh

        def zr_gates(b, gxs, gs, zs):
            gx1, gx2 = gxs[b]
            gh = gs[b]
            zrt = mid.tile([128, NJ, 2 * FO], BF16, tag="zrt")
            pzs = []
            for m in range(NJ):
                pz = psgate.tile([128, 3 * FO], F32, tag="pz")
                pzs.append(pz)
                # packed x-part covers all three gates (z|r|cand) in one rhs
                nc.tensor.matmul(pz[:], gx1[:, mslc[m]], wx1[:],
                                 start=True, stop=False)
                nc.tensor.matmul(pz[:], gx2[:, mslc[m]], wx2[:],
                                 start=False, stop=False)
                # h-part for z,r only; stop closes cols 0:256 for the sigmoid
                for P in range(NOPS):
                    nc.tensor.matmul(pz[:, 0:2 * FO], gh[:, P, mslc[m]],
                                     wh[:, P, 0:2 * FO], start=False,
                                     stop=(P == NOPS - 1),
                                     skip_group_check=(P == NOPS - 1))
                nc.scalar.activation(zrt[:, m, :], pz[:, 0:2 * FO], SIG)
            hrt = mid.tile([128, NJ, CH], BF16, tag="hrt")
            nc.vector.tensor_mul(hrt[:], hs[:, b, :, :], zrt[:, :, FO:2 * FO])
            zs[b] = (zrt, hrt, pzs)

        def cand_graph(b, zs, cs):
            _, hrt, _ = zs[b]
            gc = gcp.tile([CH, NOPS, ND], BF16, tag="gc")
            graph_ops(lambda j: hrt[:, j, :], CH, gc,
                      ["dve", "act", "dve", "act", "dve"])
            cs[b] = gc

        def cand_gates(t, b, gs, zs, cs):
            zrt, _, pzs = zs[b]
            gc = cs[b]
            ht = mid.tile([128, NJ, FO], BF16, tag="ht")
            for m in range(NJ):
                pz = pzs[m]
                for P in range(NOPS):
                    nc.tensor.matmul(pz[:, 2 * FO:], gc[:, P, mslc[m]],
                                     wh[:, P, 2 * FO:], start=False,
                                     stop=(P == NOPS - 1))
                nc.scalar.activation(ht[:, m, :], pz[:, 2 * FO:], TANH)
            d1 = spool.tile([128, NJ, FO], BF16, tag="d1")
            nc.vector.tensor_sub(d1[:], hs[:, b, :, :], ht[:])
            d2 = spool.tile([128, NJ, FO], BF16, tag="d2")
            nc.vector.tensor_mul(d2[:], zrt[:, :, 0:FO], d1[:])
            nc.vector.tensor_add(hs[:, b, :, :], ht[:], d2[:])
            nc.sync.dma_start(y_d[:, t, b, :], hs[:, b, :, :])

        for t in range(T):
            gxs, gs, zs, cs = {}, {}, {}, {}
            gx_graph(t, 0, gxs)
            zr_graph(t, 0, gs)
            zr_graph(t, 1, gs)
            zr_gates(0, gxs, gs, zs)
            for b in range(BL):
                if b == 2:
                    gx_graph(t, 1, gxs)
                if b + 2 < BL:
                    zr_graph(t, b + 2, gs)
                cand_graph(b, zs, cs)
                if b + 1 < BL:
                    zr_gates(b + 1, gxs, gs, zs)
                cand_gates(t, b, gs, zs, cs)
    nc.compile()
    return nc


def _prep_consts(edge_index, edge_weight, Wz, bz, Wr, br, Wh, bh):
    row = edge_index[0].astype(np.int64)
    col = edge_index[1].astype(np.int64)
    w = edge_weight.astype(np.float32)
    deg_out = np.zeros(N, np.float32)
    deg_in = np.zeros(N, np.float32)
    np.add.at(deg_out, row, w)
    np.add.at(deg_in, col, w)
    norm_out = (1.0 / deg_out)[row]
    norm_in = (1.0 / deg_in)[row]  # quirk: indexed by row
    perm = np.argsort(col * N + row, kind="stable")
    A_out = np.zeros((N, N), np.float32)
    A_in = np.zeros((N, N), np.float32)
    np.add.at(A_out, (col, row), norm_out)
    np.add.at(A_in, (row[perm], col[perm]), norm_in)  # norm_in unpermuted
    I = np.eye(N, dtype=np.float32)
    A_out2 = 2.0 * (A_out @ A_out) - I
    A_in2 = 2.0 * (A_in @ A_in) - I

    # rhs layout [s%128, j, P, d]: value = A_P[d, s] for s = CHUNK0[j] + srow.
    # Chunk 2 rows 0:59 (s in 197..255) duplicate chunk 1 -> zeroed.
    amat_r = np.zeros((128, NJ, NOPS, ND), np.float32)
    for i, A in enumerate([I, A_out, A_in, A_out2, A_in2]):
        for j, s0 in enumerate(CHUNK0):
            blk = A[:, s0:s0 + 128].T.copy()  # [128 srow, ND]
            if j == 2:
                blk[0:256 - 197] = 0.0
            amat_r[:, j, i, :] = blk

    def terms(W):  # W: [2, 3, C, co] -> list of 5 [C, co]
        return [W[0, 0] + W[1, 0], W[0, 1], W[1, 1], W[0, 2], W[1, 2]]

    tz, tr, th = terms(Wz), terms(Wr), terms(Wh)
    wx1 = np.zeros((KP1, 3 * F_OUT), np.float32)
    wx2 = np.zeros((KP2, 3 * F_OUT), np.float32)
    wh = np.zeros((CH, NOPS, 3 * F_OUT), np.float32)
    for P in range(NOPS):
        cat = np.concatenate([tz[P], tr[P], th[P]], axis=1)  # [C, 384]
        if P < 3:
            wx1[32 * P:32 * P + 32] = cat[:F_IN]
        else:
            wx2[32 * (P - 3):32 * (P - 3) + 32] = cat[:F_IN]
        wh[:, P] = cat[F_IN:]
    wx1[3 * F_IN] = np.concatenate([bz, br, bh])  # bias row (pairs with ones row)
    amat_r = amat_r.astype(ml_dtypes.bfloat16)
    wx1 = wx1.astype(ml_dtypes.bfloat16)
    wx2 = wx2.astype(ml_dtypes.bfloat16)
    wh = wh.astype(ml_dtypes.bfloat16)
    return np.ascontiguousarray(amat_r), wx1, wx2, wh


def kernel(X, edge_index, edge_weight, Wz, bz, Wr, br, Wh, bh):
    X = np.asarray(X, np.float32)
    amat_r, wx1, wx2, wh = _prep_consts(
        np.asarray(edge_index), np.asarray(edge_weight, np.float32),
        np.asarray(Wz, np.float32), np.asarray(bz, np.float32),
        np.asarray(Wr, np.float32), np.asarray(br, np.float32),
        np.asarray(Wh, np.float32), np.asarray(bh, np.float32))

    if "nc" not in _CACHE:
        _CACHE["nc"] = _build_bass()
    nc = _CACHE["nc"]

    in_maps = []
    for c in range(NC):
        Xl = X[c * BL:(c + 1) * BL]  # [BL, T, N, F_IN]
        Xp = np.zeros((BL, T, 128, NJ, F_IN), np.float32)
        for j, s0 in enumerate(CHUNK0):
            Xp[:, :, :, j, :] = Xl[:, :, s0:s0 + 128, :]
        # -> [p, j, t, g, bb, c] with (g, bb) = divmod(b, 4)
        Xp = Xp.reshape(NG, 4, T, 128, NJ, F_IN).transpose(3, 4, 2, 0, 1, 5)
        Xp = Xp.reshape(128, NJ, T, NG, 128)
        in_maps.append({
            "xin": np.ascontiguousarray(Xp).astype(ml_dtypes.bfloat16),
            "amat": amat_r, "wx1": wx1, "wx2": wx2, "wh": wh,
        })

    trace = bool(int(os.environ.get("KERNEL_TRACE", "0")))
    res = run_bass_kernel_spmd(nc, in_maps, core_ids=list(range(NC)), trace=trace)
    _CACHE["last_result"] = res

    out = np.empty((B, T, N, F_OUT), np.float32)
    for c in range(NC):
        y = res.results[c]["y"].astype(np.float32)  # [128, T, BL, NJ*F_OUT]
        y = y.reshape(128, T, BL, NJ, F_OUT).transpose(2, 1, 3, 0, 4)
        # [BL, T, NJ, 128, F_OUT]: nodes 0:128 | 128:256 | 256:325 (chunk2 rows 59:)
        blk = out[c * BL:(c + 1) * BL]
        blk[:, :, 0:128] = y[:, :, 0]
        blk[:, :, 128:256] = y[:, :, 1]
        blk[:, :, 256:325] = y[:, :, 2, 256 - 197:, :]
    return out


# revision 38
# speedup vs baseline: 5.7128x; 1.0717x over previous
import os
import sys
from contextlib import ExitStack

import numpy as np
import ml_dtypes

sys.path.insert(0, "/opt/trn_rl_repo")

import concourse.bass as bass
from concourse import bacc
import concourse.tile as tile
from concourse import mybir
from concourse.bass_utils import run_bass_kernel_spmd

# Problem constants (hardcoded per contract)
B, T, N, F_IN, F_OUT = 64, 12, 325, 32, 128
NC = 8          # cores
BL = B // NC    # batch per core = 8
NJ = 3          # node chunks
ND = 325        # graph output free dim (true node count, no padding)
# Overlapping node chunks: {0:128, 128:256, 197:325}. Chunk 2 rows that
# duplicate chunk 1 (nodes 197..255) are zeroed in the operator so the
# contraction over chunks stays exact, while every matmul output keeps a
# full 128-partition write (no stale-PSUM reads downstream).
CHUNK0 = [0, 128, 197]
CX = F_IN       # x channels = 32; bias rides as a packed ones-row
NG = 2          # batch groups of 4 for the packed x graph ops
CH = F_OUT      # 128
# x-side gate weights are (P, c)-packed along partitions:
#   pack1 rows: P in {0,1,2} x 32 channels = 96, plus ones/bias row -> 97
#   pack2 rows: P in {3,4} x 32 channels = 64
KP1, KP2 = 3 * F_IN + 1, 2 * F_IN
NOPS = 5        # I, A_out, A_in, A_out2, A_in2
F32 = mybir.dt.float32
BF16 = mybir.dt.bfloat16
SIG = mybir.ActivationFunctionType.Sigmoid
TANH = mybir.ActivationFunctionType.Tanh

_CACHE = {}


def _build_bass():
    nc = bacc.Bacc(None, target_bir_lowering=False)
    x_d = nc.dram_tensor("xin", [128, NJ, T, NG, 128], BF16, kind="ExternalInput")
    a_d = nc.dram_tensor("amat", [128, NJ, NOPS, ND], BF16, kind="ExternalInput")
    wx1_d = nc.dram_tensor("wx1", [KP1, 3 * F_OUT], BF16, kind="ExternalInput")
    wx2_d = nc.dram_tensor("wx2", [KP2, 3 * F_OUT], BF16, kind="ExternalInput")
    wh_d = nc.dram_tensor("wh", [CH, NOPS, 3 * F_OUT], BF16, kind="ExternalInput")
    y_d = nc.dram_tensor("y", [128, T, BL, NJ * F_OUT], BF16, kind="ExternalOutput")

    with tile.TileContext(nc) as tc, ExitStack() as ctx:
        const = ctx.enter_context(tc.tile_pool(name="const", bufs=1))
        state = ctx.enter_context(tc.tile_pool(name="state", bufs=1))
        gpool = ctx.enter_context(tc.tile_pool(name="g", bufs=3))
        gxpool = ctx.enter_context(tc.tile_pool(name="gx", bufs=6))
        gcp = ctx.enter_context(tc.tile_pool(name="gcp", bufs=2))
        fmpool = ctx.enter_context(tc.tile_pool(name="fm", bufs=10))
        mid = ctx.enter_context(tc.tile_pool(name="mid", bufs=3))
        spool = ctx.enter_context(tc.tile_pool(name="s", bufs=3))
        psg = ctx.enter_context(tc.tile_pool(name="psg", bufs=4, space="PSUM"))
        psgate = ctx.enter_context(tc.tile_pool(name="psgate", bufs=4, space="PSUM"))

        xin = const.tile([128, NJ, T, NG, 128], BF16)
        amat = const.tile([128, NJ, NOPS, ND], BF16)
        wx1 = const.tile([KP1, 3 * F_OUT], BF16)
        wx2 = const.tile([KP2, 3 * F_OUT], BF16)
        wh = const.tile([CH, NOPS, 3 * F_OUT], BF16)
        for P in range(NOPS):
            nc.sync.dma_start(amat[:, :, P, :], a_d[:, :, P, :])
        for tq in range(0, T, 3):
            nc.sync.dma_start(xin[:, :, tq:tq + 3], x_d[:, :, tq:tq + 3])
        nc.sync.dma_start(wx1[:], wx1_d[:])
        nc.sync.dma_start(wx2[:], wx2_d[:])
        nc.sync.dma_start(wh[:], wh_d[:])

        hs = state.tile([128, BL, NJ, CH], BF16)  # node-major hidden state
        nc.gpsimd.memset(hs[:], 0.0)

        FO = F_OUT
        # node-chunk slices of the graph-op output (free dim of g tiles)
        mslc = [slice(s, s + 128) for s in CHUNK0]

        def graph_ops(lhs_fn, cpart, gtile, evict_engines, p0):
            # gtile[c, P-p0, d] = sum_s lhs[s, c] * amat[s, P, d] for P=p0..4.
            # When p0=1 the identity op comes from a DMA-transposed copy of
            # the source instead, so gtile holds 4 operators only.
            for P in range(p0, NOPS):
                ps = psg.tile([128, ND], F32, tag="ps")
                for j in range(NJ):
                    nc.tensor.matmul(ps[0:cpart, :], lhs_fn(j), amat[:, j, P, :],
                                     start=(j == 0), stop=(j == NJ - 1))
                eng = evict_engines[P - p0]
                if eng == "act":
                    nc.scalar.copy(gtile[:, P - p0, :], ps[0:cpart, :])
                else:
                    nc.vector.tensor_copy(gtile[:, P - p0, :], ps[0:cpart, :])

        def gx_graph(t, g, gxs):
            # packed x graph ops for a group of 4 batches: lhsT carries
            # (4 batches x 32 channels) on its free dim, so the PE runs with
            # all 128 output partitions live. PSUM is evicted full-width once
            # per op; the per-batch (P,c) repacking runs on the idle Pool
            # engine (SBUF->SBUF).
            gx4 = gxpool.tile([128, NOPS, ND], BF16, tag="gx4")
            for P in range(NOPS):
                ps = psg.tile([128, ND], F32, tag="ps")
                for j in range(NJ):
                    nc.tensor.matmul(ps[:], xin[:, j, t, g, :], amat[:, j, P, :],
                                     start=(j == 0), stop=(j == NJ - 1))
                if P % 2 == 0:
                    nc.scalar.copy(gx4[:, P, :], ps[:])
                else:
                    nc.vector.tensor_copy(gx4[:, P, :], ps[:])
            for bb in range(4):
                g1 = gxpool.tile([KP1, ND], BF16, tag="gxp1")
                nc.gpsimd.memset(g1[3 * F_IN:KP1, :], 1.0)  # ones/bias row
                g2 = gxpool.tile([KP2, ND], BF16, tag="gxp2")
                for P in range(NOPS):
                    src = gx4[32 * bb:32 * bb + 32, P, :]
                    if P < 3:
                        nc.gpsimd.tensor_copy(g1[32 * P:32 * P + 32, :], src)
                    else:
                        nc.gpsimd.tensor_copy(g2[32 * (P - 3):32 * (P - 3) + 32, :], src)
                gxs[4 * g + bb] = (g1, g2)

        def zr_graph(t, b, gs):
            gh = gpool.tile([CH, NOPS, ND], BF16, tag="gh")
            graph_ops(lambda j: hs[:, b, j, :], CH, gh,
                      ["dve", "act", "dve", "act", "dve"], p0=0)
            gs[b] = gh

        def zr_gates(b, gxs, gs, zs, hfms):
            gx1, gx2 = gxs[b]
            gh = gs[b]
            hfm = hfms[b]
            zrt = mid.tile([128, NJ, 2 * FO], BF16, tag="zrt")
            pzs = []
            for m in range(NJ):
                pz = psgate.tile([128, 3 * FO], F32, tag="pz")
                pzs.append(pz)
                # packed x-part covers all three gates (z|r|cand) in one rhs
                nc.tensor.matmul(pz[:], gx1[:, mslc[m]], wx1[:],
                                 start=True, stop=False)
                nc.tensor.matmul(pz[:], gx2[:, mslc[m]], wx2[:],
                                 start=False, stop=False)
                # h-part for z,r only; stop closes cols 0:256 for the sigmoid
                for P in range(NOPS):
                    nc.tensor.matmul(pz[:, 0:2 * FO], gh[:, P, mslc[m]],
                                     wh[:, P, 0:2 * FO], start=False,
                                     stop=(P == NOPS - 1))
                nc.scalar.activation(zrt[:, m, :], pz[:, 0:2 * FO], SIG)
            hrt = mid.tile([128, NJ, CH], BF16, tag="hrt")
            nc.vector.tensor_mul(hrt[:], hs[:, b, :, :], zrt[:, :, FO:2 * FO])
            zs[b] = (zrt, hrt, pzs)

        def cand_graph(b, zs, cs):
            _, hrt, _ = zs[b]
            gc = gcp.tile([CH, NOPS, ND], BF16, tag="gc")
            graph_ops(lambda j: hrt[:, j, :], CH, gc,
                      ["dve", "act", "dve", "act", "dve"], p0=0)
            cs[b] = gc

        def cand_gates(t, b, gs, zs, cs, hfm_next):
            zrt, _, pzs = zs[b]
            gc = cs[b]
            ht = mid.tile([128, NJ, FO], BF16, tag="ht")
            for m in range(NJ):
                pz = pzs[m]
                for P in range(NOPS):
                    nc.tensor.matmul(pz[:, 2 * FO:], gc[:, P, mslc[m]],
                                     wh[:, P, 2 * FO:], start=False,
                                     stop=(P == NOPS - 1),
                                     skip_group_check=True)
                nc.scalar.activation(ht[:, m, :], pz[:, 2 * FO:], TANH)
            d1 = spool.tile([128, NJ, FO], BF16, tag="d1")
            nc.vector.tensor_sub(d1[:], hs[:, b, :, :], ht[:])
            d2 = spool.tile([128, NJ, FO], BF16, tag="d2")
            nc.vector.tensor_mul(d2[:], zrt[:, :, 0:FO], d1[:])
            nc.vector.tensor_add(hs[:, b, :, :], ht[:], d2[:])
            nc.sync.dma_start(y_d[:, t, b, :], hs[:, b, :, :])
            if hfm_next is not None:
                # h^T for the NEXT timestep's z/r identity term — a full
                # timestep of slack hides the DMA-transpose latency.
                hfm = fmpool.tile([CH, NJ, 128], BF16, tag="hfm")
                for j in range(NJ):
                    nc.sync.dma_start(hfm[:, j, :], hs[:, b, j, :],
                                      transpose=True)
                hfm_next[b] = hfm

        hfms = {}
        for b in range(BL):  # t=0: h=0 so h^T=0
            hfm = fmpool.tile([CH, NJ, 128], BF16, tag="hfm")
            nc.gpsimd.memset(hfm[:], 0.0)
            hfms[b] = hfm
        for t in range(T):
            gxs, gs, zs, cs = {}, {}, {}, {}
            hfm_next = {} if t + 1 < T else None
            gx_graph(t, 0, gxs)
            zr_graph(t, 0, gs)
            zr_graph(t, 1, gs)
            zr_gates(0, gxs, gs, zs, hfms)
            for b in range(BL):
                if b == 2:
                    gx_graph(t, 1, gxs)
                if b + 2 < BL:
                    zr_graph(t, b + 2, gs)
                cand_graph(b, zs, cs)
                if b + 1 < BL:
                    zr_gates(b + 1, gxs, gs, zs, hfms)
                cand_gates(t, b, gs, zs, cs, hfm_next)
            if hfm_next is not None:
                hfms = hfm_next
    nc.compile()
    return nc


def _prep_consts(edge_index, edge_weight, Wz, bz, Wr, br, Wh, bh):
    row = edge_index[0].astype(np.int64)
    col = edge_index[1].astype(np.int64)
    w = edge_weight.astype(np.float32)
    deg_out = np.zeros(N, np.float32)
    deg_in = np.zeros(N, np.float32)
    np.add.at(deg_out, row, w)
    np.add.at(deg_in, col, w)
    norm_out = (1.0 / deg_out)[row]
    norm_in = (1.0 / deg_in)[row]  # quirk: indexed by row
    perm = np.argsort(col * N + row, kind="stable")
    A_out = np.zeros((N, N), np.float32)
    A_in = np.zeros((N, N), np.float32)
    np.add.at(A_out, (col, row), norm_out)
    np.add.at(A_in, (row[perm], col[perm]), norm_in)  # norm_in unpermuted
    I = np.eye(N, dtype=np.float32)
    A_out2 = 2.0 * (A_out @ A_out) - I
    A_in2 = 2.0 * (A_in @ A_in) - I

    # rhs layout [s%128, j, P, d]: value = A_P[d, s] for s = CHUNK0[j] + srow.
    # Chunk 2 rows 0:59 (s in 197..255) duplicate chunk 1 -> zeroed.
    amat_r = np.zeros((128, NJ, NOPS, ND), np.float32)
    for i, A in enumerate([I, A_out, A_in, A_out2, A_in2]):
        for j, s0 in enumerate(CHUNK0):
            blk = A[:, s0:s0 + 128].T.copy()  # [128 srow, ND]
            if j == 2:
                blk[0:256 - 197] = 0.0
            amat_r[:, j, i, :] = blk

    def terms(W):  # W: [2, 3, C, co] -> list of 5 [C, co]
        return [W[0, 0] + W[1, 0], W[0, 1], W[1, 1], W[0, 2], W[1, 2]]

    tz, tr, th = terms(Wz), terms(Wr), terms(Wh)
    wx1 = np.zeros((KP1, 3 * F_OUT), np.float32)
    wx2 = np.zeros((KP2, 3 * F_OUT), np.float32)
    wh = np.zeros((CH, NOPS, 3 * F_OUT), np.float32)
    for P in range(NOPS):
        cat = np.concatenate([tz[P], tr[P], th[P]], axis=1)  # [C, 384]
        if P < 3:
            wx1[32 * P:32 * P + 32] = cat[:F_IN]
        else:
            wx2[32 * (P - 3):32 * (P - 3) + 32] = cat[:F_IN]
        wh[:, P] = cat[F_IN:]
    wx1[3 * F_IN] = np.concatenate([bz, br, bh])  # bias row (pairs with ones row)
    amat_r = amat_r.astype(ml_dtypes.bfloat16)
    wx1 = wx1.astype(ml_dtypes.bfloat16)
    wx2 = wx2.astype(ml_dtypes.bfloat16)
    wh = wh.astype(ml_dtypes.bfloat16)
    return np.ascontiguousarray(amat_r), wx1, wx2, wh


def kernel(X, edge_index, edge_weight, Wz, bz, Wr, br, Wh, bh):
    X = np.asarray(X, np.float32)
    amat_r, wx1, wx2, wh = _prep_consts(
        np.asarray(edge_index), np.asarray(edge_weight, np.float32),
        np.asarray(Wz, np.float32), np.asarray(bz, np.float32),
        np.asarray(Wr, np.float32), np.asarray(br, np.float32),
        np.asarray(Wh, np.float32), np.asarray(bh, np.float32))

    if "nc" not in _CACHE:
        _CACHE["nc"] = _build_bass()
    nc = _CACHE["nc"]

    in_maps = []
    for c in range(NC):
        Xl = X[c * BL:(c + 1) * BL]  # [BL, T, N, F_IN]
        Xp = np.zeros((BL, T, 128, NJ, F_IN), np.float32)
        for j, s0 in enumerate(CHUNK0):
            Xp[:, :, :, j, :] = Xl[:, :, s0:s0 + 128, :]
        # -> [p, j, t, g, bb, c] with (g, bb) = divmod(b, 4)
        Xp = Xp.reshape(NG, 4, T, 128, NJ, F_IN).transpose(3, 4, 2, 0, 1, 5)
        Xp = Xp.reshape(128, NJ, T, NG, 128)
        in_maps.append({
            "xin": np.ascontiguousarray(Xp).astype(ml_dtypes.bfloat16),
            "amat": amat_r, "wx1": wx1, "wx2": wx2, "wh": wh,
        })

    trace = bool(int(os.environ.get("KERNEL_TRACE", "0")))
    res = run_bass_kernel_spmd(nc, in_maps, core_ids=list(range(NC)), trace=trace)
    _CACHE["last_result"] = res

    out = np.empty((B, T, N, F_OUT), np.float32)
    for c in range(NC):
        y = res.results[c]["y"].astype(np.float32)  # [128, T, BL, NJ*F_OUT]
        y = y.reshape(128, T, BL, NJ, F_OUT).transpose(2, 1, 3, 0, 4)
        # [BL, T, NJ, 128, F_OUT]: nodes 0:128 | 128:256 | 256:325 (chunk2 rows 59:)
        blk = out[c * BL:(c + 1) * BL]
        blk[:, :, 0:128] = y[:, :, 0]
        blk[:, :, 128:256] = y[:, :, 1]
        blk[:, :, 256:325] = y[:, :, 2, 256 - 197:, :]
    return out
